# revision 2
# baseline (speedup 1.0000x reference)
"""nn_Att_Stack — data-parallel execution across 8 Trainium2 NeuronCores.

Sharding (per spec hint): pure data parallelism. The batch axis B=16384 is
split evenly across the 8 cores (2048 per core); the tiny parameter set is
replicated to every core. There is no cross-batch communication: each core
runs the full 3-block attention stack on its batch shard, and shards are
concatenated on the host to form the full [B, 30, 2] output.

Self-contained: only needs numpy + jax (neuron/axon backend with 8 devices).
"""
import math
import numpy as np
import jax
import jax.numpy as jnp

N = 30      # sequence length
DK = 10     # d_k
H = 2       # heads
DH = DK // H
NEG = -1e9
M = 8       # cores

LAST_EXEC_NS = None


def _linear(x, w, b):
    return x @ w.T + b


def _att_block(x, p):
    b = x.shape[0]

    def proj(w, bias):
        return _linear(x, w, bias).reshape(b, N, H, DH).transpose(0, 2, 1, 3)

    q = proj(p['wq'], p['bq'])
    k = proj(p['wk'], p['bk'])
    v = proj(p['wv'], p['bv'])
    scores = jnp.sin(jnp.einsum('bhqd,bhkd->bhqk', q, k) / math.sqrt(DH))
    mask = jnp.tril(jnp.ones((N, N), dtype=bool))  # causal
    scores = jnp.where(mask, scores, NEG)
    attn = jax.nn.softmax(scores, axis=-1)
    vals = jnp.einsum('bhqk,bhkd->bhqd', attn, v)
    concat = vals.transpose(0, 2, 1, 3).reshape(b, N, DK)
    h = jax.nn.leaky_relu(_linear(concat, p['wm1'], p['bm1']))
    return _linear(h, p['wm2'], p['bm2'])


def _norm(x, alpha, bias, eps=1e-6):
    mu = jnp.mean(x, axis=-1, keepdims=True)
    sd = jnp.std(x, axis=-1, keepdims=True, ddof=1)
    return alpha * (x - mu) / (sd + eps) + bias


def _forward(x, params):
    x = _att_block(x, params['att1'])
    x = x + _att_block(_norm(x, params['norm1_a'], params['norm1_b']), params['att2'])
    x = x + _att_block(_norm(x, params['norm2_a'], params['norm2_b']), params['att3'])
    x = _norm(x, params['norm3_a'], params['norm3_b'])
    out = jax.nn.leaky_relu(_linear(x, params['wm1'], params['bm1']))
    out = _linear(out, params['wm2'], params['bm2'])
    mean = out[..., 0:1]
    logvar = jnp.clip(out[..., 1:2], -10.0, 10.0)
    return jnp.concatenate([mean, logvar], axis=2)


_fwd = jax.pmap(_forward, axis_name='i', in_axes=(0, None))


def kernel(x, params):
    B = x.shape[0]
    assert B % M == 0
    xs = np.ascontiguousarray(np.asarray(x, dtype=np.float32).reshape(M, B // M, N, 2))
    params = jax.tree_util.tree_map(lambda a: np.asarray(a, dtype=np.float32), params)
    last_err = None
    for attempt in range(3):  # retry: transient NRT device errors observed on this fabric
        try:
            out = _fwd(xs, params)
            res = np.asarray(out).reshape(B, N, 2).astype(np.float32)
            return res
        except Exception as e:  # noqa: BLE001
            last_err = e
    raise last_err
